# revision 49
# baseline (speedup 1.0000x reference)
"""FourierLinear Trainium2 kernel v7 — mod-4 + reflection folds.

Stage 1: u_f[m] = sum_k trig(w a_f k) x[k,m], folded twice:
  parity/mod-4 fold (by a%4) as before, THEN a reflection fold
  (k' <-> range-k', the DCT/DST symmetry): cos/sin tables are even/odd
  under reflection up to a sign fixed by the a-class, so each trig side
  contracts a half-range against its own pre-reflected x operand:
    a odd:   cos x (xm[j]-xm[2048-j]),  sin x (xm[j]+xm[2048-j]),  1024->8 blk each
    a%4==0:  cos x (xA0+refl) + an extra singleton block for xA0[512],
             sin x (xA0-refl),                                     512->4(+1) blk
    a%4==2:  cos x (xA2-refl), sin x (xA2+refl),                   512->4 blk each
  Edge singletons (k'=0, mid) land in operand slot 0 / the xE0 block with
  host-overridden table rows (+-1 per lane).  Halves stage-1 PE again:
  a chunk's cos and sin streams accumulate separate psums with their own
  start/stop flags.  Mixed chunks still merge even-a group remainders.

Stage 2 (new): mod-4 l-fold.  For l' < 1024 accumulate four psum partials
per row block:
  P0 (b%4==0), P2 (b%4==2):  sum s(uc cos - us sin)(w b l')
  B  (b odd):                sum s(uc cos - us sin)
  D  (b odd):                sum sigma s(uc sin + us cos), sigma=+1 b%4==3
then with A = P0+P2, C = P0-P2:
  y[l']      = A+B   y[l'+1024] = C+D   y[l'+2048] = A-B   y[l'+3072] = C-D
Chunks must be b-class pure (b%4 in {0,2} or b odd) — the 3x3 (a-fold,
b-class) grouping plus remainder merging gives 19 chunks here vs 19 for
the v5 mod-2 scheme, but stage-2 matmul time drops 26% (even-b chunks
touch half the l' range) and the yE park/drain phase disappears (4 banks
per row block, double-buffered).  Stage-2 tables are SBUF-resident per
l'-position, loaded once and reused across all 8 row blocks.

The 2^-16 ifft2 norm (* 256) stays split: 2^-8 at the stage-1 psum copy,
2^-8 inside the stage-2 tables.
"""

import numpy as np

import concourse.mybir as mybir
import concourse.tile as tile
from concourse import bacc
from concourse.bass_utils import run_bass_kernel_spmd

N_CORES = 8
IN_F = 4096
OUT_F = 4096
NF = 2048
ROWS = 8192
M = ROWS // N_CORES   # 1024 rows per core
P = 128
KH = IN_F // 2        # 2048 folded k' (odd-a)
KQ = IN_F // 4        # 1024 quarter-folded k' (even-a)
LQ = OUT_F // 4       # 1024 quarter-folded l' range
NT = 512
NPOS = LQ // NT       # 2 l'-positions
MS = M // P           # 8 row blocks
KCH_MAX = KH // P     # 16

XBLK = 33     # folded-x operand blocks: xA0c xA0s xA2c xA2s xmc xms xE0
TMAX = 17     # max table blocks per normal chunk
TBIG = 33     # merged (even+odd) chunks carry up to all three set types

LAST_RESULTS = None
_NC_CACHE = None


def _build_nc(desc):
    # desc: tuple of (bcl, sets) per 128-lane f-chunk, sorted bcl-major.
    #   bcl: 0 -> b%4==0 (P0), 1 -> b%4==2 (P2), 2 -> b odd (B and D)
    #   sets: tuple of (kch, xsel); xsel 0 -> xA0, 1 -> xA2, 2 -> xm
    NCH = len(desc)
    odd_ids = [i for i, (bcl, _) in enumerate(desc) if bcl == 2]
    NODD = len(odd_ids)
    oidx = {ci: k for k, ci in enumerate(odd_ids)}
    koff = [0]
    for _, streams in desc:
        koff.append(koff[-1] + sum(n for n, _, _ in streams))
    TBLK = koff[-1]
    # per-psum-class chunk ranges (for matmul start/stop flags)
    cls_ids = {c: [i for i, (bcl, _) in enumerate(desc) if bcl == c]
               for c in range(3)}
    f32 = mybir.dt.float32
    f16 = mybir.dt.float16
    mult = mybir.AluOpType.mult
    add = mybir.AluOpType.add
    sub = mybir.AluOpType.subtract

    nc = bacc.Bacc(None)
    xop = nc.declare_dram_parameter("xop", [XBLK * P, M], f16, isOutput=False)
    t1c = nc.declare_dram_parameter("t1c", [TBLK * P, P], f16, isOutput=False)
    t2pc = nc.declare_dram_parameter("t2pc", [NCH * NPOS * P, NT], f16,
                                     isOutput=False)
    t2ps = nc.declare_dram_parameter("t2ps", [NCH * NPOS * P, NT], f16,
                                     isOutput=False)
    t2dc = nc.declare_dram_parameter("t2dc", [NODD * NPOS * P, NT], f16,
                                     isOutput=False)
    t2ds = nc.declare_dram_parameter("t2ds", [NODD * NPOS * P, NT], f16,
                                     isOutput=False)
    out = nc.declare_dram_parameter("out", [M, OUT_F], f16, isOutput=True)

    xopp = xop[:].rearrange("(kc p) m -> p kc m", p=P)
    t1cp = t1c[:].rearrange("(blk p) j -> p blk j", p=P)
    t2pcp = t2pc[:].rearrange("(ch ps p) l -> p ch ps l", ch=NCH, ps=NPOS, p=P)
    t2psp = t2ps[:].rearrange("(ch ps p) l -> p ch ps l", ch=NCH, ps=NPOS, p=P)
    t2dcp = t2dc[:].rearrange("(ch ps p) l -> p ch ps l", ch=NODD, ps=NPOS, p=P)
    t2dsp = t2ds[:].rearrange("(ch ps p) l -> p ch ps l", ch=NODD, ps=NPOS, p=P)
    outp = out[:].rearrange("(ms p) n -> p ms n", p=P)

    with tile.TileContext(nc) as tc:
        with (
            tc.tile_pool(name="v", bufs=1) as vpool,
            tc.tile_pool(name="o", bufs=2) as opool,
        ):
            vc = vpool.tile([P, NCH, M], f16)   # u_cos * 2^-8
            vs = vpool.tile([P, NCH, M], f16)
            # pos-0 stage-2 tables: allocated outside the stage-1 pools and
            # DMA'd behind stage-1's queue traffic, so they are resident
            # before the stage transition (fits now that stage-1 streams a
            # single table tensor)
            tpa0 = vpool.tile([P, NCH, NT], f16, tag="tpa0")
            tpb0 = vpool.tile([P, NCH, NT], f16, tag="tpb0")
            # ---- stage 1
            with (
                tc.tile_pool(name="x", bufs=1) as xpool,
                tc.tile_pool(name="t1", bufs=3) as t1p,
                tc.tile_pool(name="t1b", bufs=1) as t1b,
                tc.tile_pool(name="ps1", bufs=4, space="PSUM") as ps1,
            ):
                xt = xpool.tile([P, XBLK, M], f16)   # all folded x operands

                def tbl_tiles(i):
                    # whole table on ONE alternating queue: a chunk then
                    # depends on a single queue's progress instead of the
                    # max of both (halves split across queues coupled every
                    # chunk to the more backlogged queue)
                    tot = koff[i + 1] - koff[i]
                    if tot > TMAX:
                        tcb = t1b.tile([P, TBIG, P], f16, tag="tb")
                    else:
                        tcb = t1p.tile([P, TMAX, P], f16, tag="t")
                    eng = nc.sync if i % 2 == 0 else nc.scalar
                    eng.dma_start(
                        tcb[:, 0:tot, :], t1cp[:, koff[i] : koff[i] + tot, :])
                    return tcb

                # x-piece DMAs in first-use order, emitted interleaved with
                # the per-chunk table DMAs: queue FIFO order then delivers
                # chunk i+1's tables between x pieces instead of parking all
                # table traffic behind the full 8 MB x stream.
                NPRE = 2
                pre = [tbl_tiles(i) for i in range(NPRE)]
                xq = [8, 9, 10, 11, 12, 13, 14, 15, 0, 1, 2, 3, 32, 4, 5, 6, 7]
                xq += list(range(16, 32))
                qi = 0

                def emit_x(n):
                    nonlocal qi
                    for _ in range(min(n, len(xq))):
                        kc = xq.pop(0)
                        eng = nc.sync if qi % 2 == 0 else nc.scalar
                        eng.dma_start(xt[:, kc, :], xopp[:, kc, :])
                        qi += 1

                emit_x(12)
                for i, (bcl, streams) in enumerate(desc):
                    tcb = pre[i] if i < NPRE else tbl_tiles(i)
                    emit_x(3)
                    psc = ps1.tile([P, M], f32, tag="u", name=f"psc{i}")
                    pss = ps1.tile([P, M], f32, tag="u", name=f"pss{i}")
                    ncos = sum(n for n, _, t in streams if t == 0)
                    nsin = sum(n for n, _, t in streams if t == 1)
                    ic = isn = bi = 0
                    for nblk, xb0, tgt in streams:
                        for b in range(nblk):
                            if tgt == 0:
                                ps, st, sp = psc, ic == 0, ic == ncos - 1
                                ic += 1
                            else:
                                ps, st, sp = pss, isn == 0, isn == nsin - 1
                                isn += 1
                            nc.tensor.matmul(ps[:, 0:NT], tcb[:, bi, :],
                                             xt[:, xb0 + b, 0:NT],
                                             start=st, stop=sp)
                            nc.tensor.matmul(ps[:, NT:M], tcb[:, bi, :],
                                             xt[:, xb0 + b, NT:M],
                                             start=st, stop=sp)
                            bi += 1
                    nc.scalar.mul(vc[:, i, :], psc[:], 2.0 ** -8)
                    nc.vector.tensor_scalar(vs[:, i, :], pss[:], 2.0 ** -8,
                                            None, mult)
                for ci in range(NCH):
                    nc.sync.dma_start(tpa0[:, ci, :], t2pcp[:, ci, 0, :])
                    nc.scalar.dma_start(tpb0[:, ci, :], t2psp[:, ci, 0, :])

            # ---- stage 2: four partials per (pos, ms), combine, write 4 blocks
            with (
                tc.tile_pool(name="t2p", bufs=1) as t2pp,
                tc.tile_pool(name="t2d", bufs=1) as t2dp,
                tc.tile_pool(name="ac", bufs=2) as acp,
                tc.tile_pool(name="ps2", bufs=2, space="PSUM") as ps2,
            ):
                for pos in range(NPOS):
                    # per-chunk sliced loads in consumption order: matmuls
                    # start as soon as the first chunks land instead of
                    # waiting for one monolithic multi-MB transfer (which
                    # also sits behind the WAR on the freed stage-1 region)
                    if pos == 0:
                        tpa, tpb = tpa0, tpb0
                    else:
                        tpa = t2pp.tile([P, NCH, NT], f16, tag="tpa")
                        tpb = t2pp.tile([P, NCH, NT], f16, tag="tpb")
                        for ci in range(NCH):
                            nc.sync.dma_start(tpa[:, ci, :],
                                              t2pcp[:, ci, pos, :])
                            nc.scalar.dma_start(tpb[:, ci, :],
                                                t2psp[:, ci, pos, :])
                    tda = t2dp.tile([P, NODD, NT], f16, tag="tda")
                    tdb = t2dp.tile([P, NODD, NT], f16, tag="tdb")
                    for oi in range(NODD):
                        nc.gpsimd.dma_start(tda[:, oi, :], t2dcp[:, oi, pos, :])
                        nc.gpsimd.dma_start(tdb[:, oi, :], t2dsp[:, oi, pos, :])
                    for ms in range(MS):
                        pp = [ps2.tile([P, NT], f32, tag=f"pp{c}",
                                       name=f"pp{pos}_{ms}_{c}")
                              for c in range(3)]
                        dd = ps2.tile([P, NT], f32, tag="dd",
                                      name=f"dd{pos}_{ms}")
                        msl = slice(ms * P, (ms + 1) * P)
                        # class-major order: P0/P2 stop early so the DVE
                        # A/C prep overlaps the odd-class matmuls (tables
                        # are resident: pos 0 prestaged, pos 1 prefetched)
                        for ci in cls_ids[0] + cls_ids[1] + cls_ids[2]:
                            c = desc[ci][0]
                            ids = cls_ids[c]
                            nc.tensor.matmul(
                                pp[c][:], vc[:, ci, msl], tpa[:, ci, :],
                                start=(ci == ids[0]), stop=False)
                            if c == 2:
                                oi = oidx[ci]
                                nc.tensor.matmul(
                                    dd[:], vc[:, ci, msl], tda[:, oi, :],
                                    start=(oi == 0), stop=False)
                            nc.tensor.matmul(
                                pp[c][:], vs[:, ci, msl], tpb[:, ci, :],
                                start=False, stop=(ci == ids[-1]))
                            if c == 2:
                                nc.tensor.matmul(
                                    dd[:], vs[:, ci, msl], tdb[:, oi, :],
                                    start=False, stop=(oi == NODD - 1))
                        # A = P0+P2, C = P0-P2; y_j = A+-B / C+-D.  DVE may
                        # read only one PSUM operand, so P0 goes to SBUF
                        # first (on the otherwise-idle scalar engine).
                        p0sb = acp.tile([P, NT], f32, tag="p0", name="p0sb")
                        at = acp.tile([P, NT], f32, tag="A", name="at")
                        ct = acp.tile([P, NT], f32, tag="C", name="ct")
                        nc.scalar.copy(out=p0sb[:], in_=pp[0][:])
                        nc.vector.tensor_tensor(out=at[:], in0=p0sb[:],
                                                in1=pp[1][:], op=add)
                        nc.vector.tensor_tensor(out=ct[:], in0=p0sb[:],
                                                in1=pp[1][:], op=sub)
                        for j, (lhs, ps, op) in enumerate(
                            ((at, pp[2], add), (ct, dd, add),
                             (at, pp[2], sub), (ct, dd, sub))):
                            ot = opool.tile([P, NT], f16, tag=f"y{j}",
                                            name=f"y{j}")
                            nc.vector.tensor_tensor(out=ot[:], in0=lhs[:],
                                                    in1=ps[:], op=op)
                            col = pos * NT + LQ * j
                            eng = nc.scalar if j % 2 == 0 else nc.sync
                            eng.dma_start(outp[:, ms, col : col + NT], ot[:])
    nc.finalize()
    return nc


def _host_prep(x, spectrum, indices):
    x2 = np.asarray(x, dtype=np.float32).reshape(ROWS, IN_F)
    idx = np.asarray(indices, dtype=np.int64)
    s = np.asarray(spectrum, dtype=np.float32)
    a, b = idx[0], idx[1]

    # reference scatter is last-write-wins on duplicate (a,b) pairs
    keys = a * OUT_F + b
    _, first_of_reversed = np.unique(keys[::-1], return_index=True)
    keep = np.zeros(NF, dtype=bool)
    keep[NF - 1 - first_of_reversed] = True
    s_eff = np.where(keep, s, 0.0).astype(np.float32)

    # per-set matmul streams: (n table blocks, xop block offset, 0=cos/1=sin)
    SET_STREAMS = {
        0: ((4, 0, 0), (1, 32, 0), (4, 4, 1)),    # a%4==0: xA0c, xE0, xA0s
        1: ((4, 8, 0), (4, 12, 1)),               # a%4==2: xA2c, xA2s
        2: ((8, 16, 0), (8, 24, 1)),              # a odd:  xmc,  xms
    }
    # chunks: 3x3 (a-fold, b-class) groups, even-a remainders merged per
    # b-class into mixed chunks; even-a chunks first, odd-a last (the xm
    # operands stream later); stage-2 class membership is an explicit list
    afold = [(lambda v: v % 4 == 0, 0, 0), (lambda v: v % 4 == 2, 1, 2),
             (lambda v: v % 2 == 1, 2, 1)]
    bklass = [(lambda v: v % 4 == 0, 0), (lambda v: v % 4 == 2, 2),
              (lambda v: v % 2 == 1, 1)]
    chunk_f, chunk_xsel, desc = [], [], []
    odd_f, odd_xsel, odd_desc = [], [], []

    def mkstreams(xsi):
        st = ()
        for xs in sorted(set(xsi)):
            st = st + SET_STREAMS[xs]
        return st

    for bcl, (bsel, db) in enumerate(bklass):
        pool_f, pool_xs = [], []
        for asel, xsel, da in afold[:2]:
            sel = np.nonzero(asel(a) & bsel(b))[0]
            nfull = len(sel) // P
            for c in range(nfull):
                chunk_f.append(sel[c * P : (c + 1) * P])
                chunk_xsel.append([xsel] * P)
                desc.append((bcl, SET_STREAMS[xsel]))
            pool_f += list(sel[nfull * P :])
            pool_xs += [xsel] * (len(sel) - nfull * P)
        mixed = [(pool_f[c0 : c0 + P], pool_xs[c0 : c0 + P])
                 for c0 in range(0, len(pool_f), P)]
        asel, xsel, da = afold[2]
        sel = np.nonzero(asel(a) & bsel(b))[0]
        nfull = len(sel) // P
        rem = list(sel[nfull * P :])
        # merge the odd-a remainder into a partial even chunk of the SAME
        # b-class (stage-2 only needs b-class purity): saves a whole chunk
        # of stage-2 matmuls at zero stage-1 cost
        if rem and mixed and len(mixed[-1][0]) + len(rem) <= P:
            fi, xsi = mixed.pop()
            fi = fi + rem
            xsi = xsi + [2] * len(rem)
            pad = P - len(fi)
            odd_f.append(np.array(fi + [-1] * pad))
            odd_xsel.append(xsi + [xsi[0]] * pad)
            odd_desc.append((bcl, mkstreams(xsi)))
            rem = []
        for fi, xsi in mixed:
            pad = P - len(fi)
            chunk_f.append(np.array(fi + [-1] * pad))
            chunk_xsel.append(xsi + [xsi[0]] * pad)
            desc.append((bcl, mkstreams(xsi)))
        for c in range(nfull):
            odd_f.append(sel[c * P : (c + 1) * P])
            odd_xsel.append([2] * P)
            odd_desc.append((bcl, SET_STREAMS[2]))
        if rem or (nfull == 0 and not any(d[0] == bcl for d in odd_desc)
                   and not any(d[0] == bcl for d in desc)):
            pad = P - len(rem)
            odd_f.append(np.array(rem + [-1] * pad))
            odd_xsel.append([2] * P)
            odd_desc.append((bcl, SET_STREAMS[2]))
    def ekey(d):
        if d[1] == SET_STREAMS[1]:
            return 0
        if d[1] == SET_STREAMS[0]:
            return 1
        return 2
    ez = sorted(zip(chunk_f, chunk_xsel, desc), key=lambda z: ekey(z[2]))
    chunk_f = [z[0] for z in ez] + odd_f
    chunk_xsel = [z[1] for z in ez] + odd_xsel
    desc = [z[2] for z in ez] + odd_desc
    # padded lane arrays; dummy lanes (f == -1) get parity-consistent a/b
    dummy_b = {0: 0, 1: 2, 2: 1}
    dummy_a = {0: 0, 1: 2, 2: 1}
    NCH = len(desc)
    a2 = np.zeros(NCH * P, np.int64)
    b2 = np.zeros(NCH * P, np.int64)
    s2 = np.zeros(NCH * P, np.float32)
    for i in range(NCH):
        bcl = desc[i][0]
        fi = chunk_f[i]
        for j in range(P):
            if fi[j] >= 0:
                a2[i * P + j] = a[fi[j]]
                b2[i * P + j] = b[fi[j]]
                s2[i * P + j] = s_eff[fi[j]]
            else:
                a2[i * P + j] = dummy_a[chunk_xsel[i][j]]
                b2[i * P + j] = dummy_b[bcl]

    w = 2.0 * np.pi / 4096.0
    # stage-1 tables: per chunk, per stream, cos/sin of the LOCAL operand
    # index; singleton terms live in host-overridden rows (see xop build)
    xgrp = {0: 0, 4: 0, 32: 0, 8: 1, 12: 1, 16: 2, 24: 2}
    t1_parts = []
    for i, (bcl, streams) in enumerate(desc):
        al = a2[i * P : (i + 1) * P]
        mask = None
        xsl = np.array(chunk_xsel[i])
        for nblk, xb0, tgt in streams:
            m = (xsl == xgrp[xb0])[None, :]
            if xb0 == 32:
                blk = np.zeros((P, P), np.float32)
                blk[0] = np.where(m[0], (-1.0) ** ((al // 4) % 2), 0.0)
                t1_parts.append(blk)
                continue
            jj = np.arange(nblk * P)
            ph = (al[None, :] * jj[:, None]) % 4096
            tb = (np.cos(w * ph, dtype=np.float32) if tgt == 0
                  else np.sin(w * ph, dtype=np.float32))
            if tgt == 1 and xb0 == 12:
                tb[0] = (-1.0) ** (((al - 2) // 4) % 2)
            if tgt == 1 and xb0 == 24:
                tb[0] = (-1.0) ** (((al - 1) // 2) % 2)
            t1_parts.append(np.where(m, tb, 0.0).astype(np.float32))
    t1c = np.ascontiguousarray(
        np.concatenate(t1_parts, axis=0).astype(np.float16))

    # stage-2 tables over l' < 1024, s*2^-8 folded in
    ll = np.arange(LQ)
    odd_ids = [i for i, (bcl, _) in enumerate(desc) if bcl == 2]
    NODD = len(odd_ids)
    t2pc = np.zeros((NCH, NPOS, P, NT), np.float32)
    t2ps = np.zeros((NCH, NPOS, P, NT), np.float32)
    t2dc = np.zeros((NODD, NPOS, P, NT), np.float32)
    t2ds = np.zeros((NODD, NPOS, P, NT), np.float32)
    oi = 0
    for i, (bcl, _) in enumerate(desc):
        bl = b2[i * P : (i + 1) * P]
        sc = (s2[i * P : (i + 1) * P] * 2.0 ** -8)[:, None]
        ph = (bl[:, None] * ll[None, :]) % 4096
        cosb = np.cos(w * ph, dtype=np.float32)
        sinb = np.sin(w * ph, dtype=np.float32)
        t2pc[i] = (cosb * sc).reshape(P, NPOS, NT).transpose(1, 0, 2)
        t2ps[i] = (-sinb * sc).reshape(P, NPOS, NT).transpose(1, 0, 2)
        if bcl == 2:
            sg = np.where(bl % 4 == 3, 1.0, -1.0)[:, None]
            t2dc[oi] = (sinb * sc * sg).reshape(P, NPOS, NT).transpose(1, 0, 2)
            t2ds[oi] = (cosb * sc * sg).reshape(P, NPOS, NT).transpose(1, 0, 2)
            oi += 1
    t2pc = np.ascontiguousarray(t2pc.reshape(NCH * NPOS * P, NT).astype(np.float16))
    t2ps = np.ascontiguousarray(t2ps.reshape(NCH * NPOS * P, NT).astype(np.float16))
    t2dc = np.ascontiguousarray(t2dc.reshape(NODD * NPOS * P, NT).astype(np.float16))
    t2ds = np.ascontiguousarray(t2ds.reshape(NODD * NPOS * P, NT).astype(np.float16))

    # folded x operands: parity folds then reflection folds (singletons in
    # slot 0 / the xE0 block, matching the table-row overrides above)
    q0, q1, q2, q3 = (x2[:, i * KQ : (i + 1) * KQ] for i in range(4))
    xA0 = q0 + q1 + q2 + q3
    xA2 = q0 - q1 + q2 - q3
    xm = x2[:, :KH] - x2[:, KH:]
    rA0 = xA0[:, ::-1]
    rA2 = xA2[:, ::-1]
    rm = xm[:, ::-1]
    half = KQ // 2   # 512
    xA0c = np.concatenate([xA0[:, :1], xA0[:, 1:half] + rA0[:, 0:half - 1]], 1)
    xA0s = np.concatenate([np.zeros_like(xA0[:, :1]),
                           xA0[:, 1:half] - rA0[:, 0:half - 1]], 1)
    xA2c = np.concatenate([xA2[:, :1], xA2[:, 1:half] - rA2[:, 0:half - 1]], 1)
    xA2s = np.concatenate([xA2[:, half : half + 1],
                           xA2[:, 1:half] + rA2[:, 0:half - 1]], 1)
    xmc = np.concatenate([xm[:, :1], xm[:, 1:KQ] - rm[:, 0:KQ - 1]], 1)
    xms = np.concatenate([xm[:, KQ : KQ + 1], xm[:, 1:KQ] + rm[:, 0:KQ - 1]], 1)
    xE0 = np.zeros((ROWS, P), np.float32)
    xE0[:, 0] = xA0[:, half]
    xop = np.concatenate(
        [xA0c, xA0s, xA2c, xA2s, xmc, xms, xE0], axis=1).astype(np.float16)
    tabs = {"t1c": t1c, "t2pc": t2pc, "t2ps": t2ps,
            "t2dc": t2dc, "t2ds": t2ds}
    return xop, tabs, tuple(desc)


def kernel(x, spectrum, indices):
    global _NC_CACHE, LAST_RESULTS
    xop, tabs, desc = _host_prep(x, spectrum, indices)

    if _NC_CACHE is None or _NC_CACHE[0] != desc:
        _NC_CACHE = (desc, _build_nc(desc))
    nc = _NC_CACHE[1]

    in_maps = [
        {
            "xop": np.ascontiguousarray(xop[j * M : (j + 1) * M].T),
            **tabs,
        }
        for j in range(N_CORES)
    ]
    res = run_bass_kernel_spmd(nc, in_maps, list(range(N_CORES)))
    LAST_RESULTS = res
    out = np.concatenate(
        [res.results[j]["out"].astype(np.float32) for j in range(N_CORES)], axis=0
    )
    return out.reshape(np.asarray(x).shape[:-1] + (OUT_F,))


# revision 50
# speedup vs baseline: 1.0718x; 1.0718x over previous
"""FourierLinear Trainium2 kernel v7 — mod-4 + reflection folds.

Stage 1: u_f[m] = sum_k trig(w a_f k) x[k,m], folded twice:
  parity/mod-4 fold (by a%4) as before, THEN a reflection fold
  (k' <-> range-k', the DCT/DST symmetry): cos/sin tables are even/odd
  under reflection up to a sign fixed by the a-class, so each trig side
  contracts a half-range against its own pre-reflected x operand:
    a odd:   cos x (xm[j]-xm[2048-j]),  sin x (xm[j]+xm[2048-j]),  1024->8 blk each
    a%4==0:  cos x (xA0+refl) + an extra singleton block for xA0[512],
             sin x (xA0-refl),                                     512->4(+1) blk
    a%4==2:  cos x (xA2-refl), sin x (xA2+refl),                   512->4 blk each
  Edge singletons (k'=0, mid) land in operand slot 0 / the xE0 block with
  host-overridden table rows (+-1 per lane).  Halves stage-1 PE again:
  a chunk's cos and sin streams accumulate separate psums with their own
  start/stop flags.  Mixed chunks still merge even-a group remainders.

Stage 2 (new): mod-4 l-fold.  For l' < 1024 accumulate four psum partials
per row block:
  P0 (b%4==0), P2 (b%4==2):  sum s(uc cos - us sin)(w b l')
  B  (b odd):                sum s(uc cos - us sin)
  D  (b odd):                sum sigma s(uc sin + us cos), sigma=+1 b%4==3
then with A = P0+P2, C = P0-P2:
  y[l']      = A+B   y[l'+1024] = C+D   y[l'+2048] = A-B   y[l'+3072] = C-D
Chunks must be b-class pure (b%4 in {0,2} or b odd) — the 3x3 (a-fold,
b-class) grouping plus remainder merging gives 19 chunks here vs 19 for
the v5 mod-2 scheme, but stage-2 matmul time drops 26% (even-b chunks
touch half the l' range) and the yE park/drain phase disappears (4 banks
per row block, double-buffered).  Stage-2 tables are SBUF-resident per
l'-position, loaded once and reused across all 8 row blocks.

The 2^-16 ifft2 norm (* 256) stays split: 2^-8 at the stage-1 psum copy,
2^-8 inside the stage-2 tables.
"""

import numpy as np

import concourse.mybir as mybir
import concourse.tile as tile
from concourse import bacc
from concourse.bass_utils import run_bass_kernel_spmd

N_CORES = 8
IN_F = 4096
OUT_F = 4096
NF = 2048
ROWS = 8192
M = ROWS // N_CORES   # 1024 rows per core
P = 128
KH = IN_F // 2        # 2048 folded k' (odd-a)
KQ = IN_F // 4        # 1024 quarter-folded k' (even-a)
LQ = OUT_F // 4       # 1024 quarter-folded l' range
NT = 512
NPOS = LQ // NT       # 2 l'-positions
MS = M // P           # 8 row blocks
KCH_MAX = KH // P     # 16

XBLK = 33     # folded-x operand blocks: xA0c xA0s xA2c xA2s xmc xms xE0
TMAX = 17     # max table blocks per normal chunk
TBIG = 33     # merged (even+odd) chunks carry up to all three set types

LAST_RESULTS = None
_NC_CACHE = None


def _build_nc(desc):
    # desc: tuple of (bcl, sets) per 128-lane f-chunk, sorted bcl-major.
    #   bcl: 0 -> b%4==0 (P0), 1 -> b%4==2 (P2), 2 -> b odd (B and D)
    #   sets: tuple of (kch, xsel); xsel 0 -> xA0, 1 -> xA2, 2 -> xm
    NCH = len(desc)
    odd_ids = [i for i, (bcl, _) in enumerate(desc) if bcl == 2]
    NODD = len(odd_ids)
    oidx = {ci: k for k, ci in enumerate(odd_ids)}
    koff = [0]
    for _, streams in desc:
        koff.append(koff[-1] + sum(n for n, _, _ in streams))
    TBLK = koff[-1]
    # per-psum-class chunk ranges (for matmul start/stop flags)
    cls_ids = {c: [i for i, (bcl, _) in enumerate(desc) if bcl == c]
               for c in range(3)}
    f32 = mybir.dt.float32
    f16 = mybir.dt.float16
    mult = mybir.AluOpType.mult
    add = mybir.AluOpType.add
    sub = mybir.AluOpType.subtract

    nc = bacc.Bacc(None)
    xop = nc.declare_dram_parameter("xop", [XBLK * P, M], f16, isOutput=False)
    t1c = nc.declare_dram_parameter("t1c", [TBLK * P, P], f16, isOutput=False)
    t2pc = nc.declare_dram_parameter("t2pc", [NCH * NPOS * P, NT], f16,
                                     isOutput=False)
    t2ps = nc.declare_dram_parameter("t2ps", [NCH * NPOS * P, NT], f16,
                                     isOutput=False)
    t2dc = nc.declare_dram_parameter("t2dc", [NODD * NPOS * P, NT], f16,
                                     isOutput=False)
    t2ds = nc.declare_dram_parameter("t2ds", [NODD * NPOS * P, NT], f16,
                                     isOutput=False)
    out = nc.declare_dram_parameter("out", [M, OUT_F], f16, isOutput=True)

    xopp = xop[:].rearrange("(kc p) m -> p kc m", p=P)
    t1cp = t1c[:].rearrange("(blk p) j -> p blk j", p=P)
    t2pcp = t2pc[:].rearrange("(ch ps p) l -> p ch ps l", ch=NCH, ps=NPOS, p=P)
    t2psp = t2ps[:].rearrange("(ch ps p) l -> p ch ps l", ch=NCH, ps=NPOS, p=P)
    t2dcp = t2dc[:].rearrange("(ch ps p) l -> p ch ps l", ch=NODD, ps=NPOS, p=P)
    t2dsp = t2ds[:].rearrange("(ch ps p) l -> p ch ps l", ch=NODD, ps=NPOS, p=P)
    outp = out[:].rearrange("(ms p) n -> p ms n", p=P)

    with tile.TileContext(nc) as tc:
        with (
            tc.tile_pool(name="v", bufs=1) as vpool,
            tc.tile_pool(name="o", bufs=2) as opool,
        ):
            vc = vpool.tile([P, NCH, M], f16)   # u_cos * 2^-8
            vs = vpool.tile([P, NCH, M], f16)
            # pos-0 stage-2 tables: allocated outside the stage-1 pools and
            # DMA'd behind stage-1's queue traffic, so they are resident
            # before the stage transition (fits now that stage-1 streams a
            # single table tensor)
            tpa0 = vpool.tile([P, NCH, NT], f16, tag="tpa0")
            tpb0 = vpool.tile([P, NCH, NT], f16, tag="tpb0")
            # ---- stage 1
            with (
                tc.tile_pool(name="x", bufs=1) as xpool,
                tc.tile_pool(name="t1", bufs=3) as t1p,
                tc.tile_pool(name="t1b", bufs=1) as t1b,
                tc.tile_pool(name="ps1", bufs=4, space="PSUM") as ps1,
            ):
                xt = xpool.tile([P, XBLK, M], f16)   # all folded x operands

                def tbl_tiles(i):
                    tot = koff[i + 1] - koff[i]
                    h = (tot + 1) // 2
                    if tot > TMAX:
                        tcb = t1b.tile([P, TBIG, P], f16, tag="tb")
                    else:
                        tcb = t1p.tile([P, TMAX, P], f16, tag="t")
                    nc.sync.dma_start(
                        tcb[:, 0:h, :], t1cp[:, koff[i] : koff[i] + h, :])
                    nc.scalar.dma_start(
                        tcb[:, h:tot, :], t1cp[:, koff[i] + h : koff[i] + tot, :])
                    return tcb

                # x-piece DMAs in first-use order, emitted interleaved with
                # the per-chunk table DMAs: queue FIFO order then delivers
                # chunk i+1's tables between x pieces instead of parking all
                # table traffic behind the full 8 MB x stream.
                NPRE = 2
                pre = [tbl_tiles(i) for i in range(NPRE)]
                xq = [8, 9, 10, 11, 12, 13, 14, 15, 0, 1, 2, 3, 32, 4, 5, 6, 7]
                xq += list(range(16, 32))
                qi = 0

                def emit_x(n):
                    nonlocal qi
                    for _ in range(min(n, len(xq))):
                        kc = xq.pop(0)
                        eng = nc.sync if qi % 2 == 0 else nc.scalar
                        eng.dma_start(xt[:, kc, :], xopp[:, kc, :])
                        qi += 1

                emit_x(12)
                for i, (bcl, streams) in enumerate(desc):
                    tcb = pre[i] if i < NPRE else tbl_tiles(i)
                    emit_x(3)
                    psc = ps1.tile([P, M], f32, tag="u", name=f"psc{i}")
                    pss = ps1.tile([P, M], f32, tag="u", name=f"pss{i}")
                    ncos = sum(n for n, _, t in streams if t == 0)
                    nsin = sum(n for n, _, t in streams if t == 1)
                    ic = isn = bi = 0
                    for nblk, xb0, tgt in streams:
                        for b in range(nblk):
                            if tgt == 0:
                                ps, st, sp = psc, ic == 0, ic == ncos - 1
                                ic += 1
                            else:
                                ps, st, sp = pss, isn == 0, isn == nsin - 1
                                isn += 1
                            nc.tensor.matmul(ps[:, 0:NT], tcb[:, bi, :],
                                             xt[:, xb0 + b, 0:NT],
                                             start=st, stop=sp)
                            nc.tensor.matmul(ps[:, NT:M], tcb[:, bi, :],
                                             xt[:, xb0 + b, NT:M],
                                             start=st, stop=sp)
                            bi += 1
                    nc.scalar.mul(vc[:, i, :], psc[:], 2.0 ** -8)
                    nc.vector.tensor_scalar(vs[:, i, :], pss[:], 2.0 ** -8,
                                            None, mult)
                for ci in range(NCH):
                    nc.sync.dma_start(tpa0[:, ci, :], t2pcp[:, ci, 0, :])
                    nc.scalar.dma_start(tpb0[:, ci, :], t2psp[:, ci, 0, :])

            # ---- stage 2: four partials per (pos, ms), combine, write 4 blocks
            with (
                tc.tile_pool(name="t2p", bufs=1) as t2pp,
                tc.tile_pool(name="t2d", bufs=1) as t2dp,
                tc.tile_pool(name="ac", bufs=2) as acp,
                tc.tile_pool(name="ps2", bufs=2, space="PSUM") as ps2,
            ):
                for pos in range(NPOS):
                    # per-chunk sliced loads in consumption order: matmuls
                    # start as soon as the first chunks land instead of
                    # waiting for one monolithic multi-MB transfer (which
                    # also sits behind the WAR on the freed stage-1 region)
                    if pos == 0:
                        tpa, tpb = tpa0, tpb0
                    else:
                        tpa = t2pp.tile([P, NCH, NT], f16, tag="tpa")
                        tpb = t2pp.tile([P, NCH, NT], f16, tag="tpb")
                        for ci in range(NCH):
                            nc.sync.dma_start(tpa[:, ci, :],
                                              t2pcp[:, ci, pos, :])
                            nc.scalar.dma_start(tpb[:, ci, :],
                                                t2psp[:, ci, pos, :])
                    tda = t2dp.tile([P, NODD, NT], f16, tag="tda")
                    tdb = t2dp.tile([P, NODD, NT], f16, tag="tdb")
                    for oi in range(NODD):
                        nc.gpsimd.dma_start(tda[:, oi, :], t2dcp[:, oi, pos, :])
                        nc.gpsimd.dma_start(tdb[:, oi, :], t2dsp[:, oi, pos, :])
                    for ms in range(MS):
                        pp = [ps2.tile([P, NT], f32, tag=f"pp{c}",
                                       name=f"pp{pos}_{ms}_{c}")
                              for c in range(3)]
                        dd = ps2.tile([P, NT], f32, tag="dd",
                                      name=f"dd{pos}_{ms}")
                        msl = slice(ms * P, (ms + 1) * P)
                        # class-major order: P0/P2 stop early so the DVE
                        # A/C prep overlaps the odd-class matmuls (tables
                        # are resident: pos 0 prestaged, pos 1 prefetched)
                        for ci in cls_ids[0] + cls_ids[1] + cls_ids[2]:
                            c = desc[ci][0]
                            ids = cls_ids[c]
                            nc.tensor.matmul(
                                pp[c][:], vc[:, ci, msl], tpa[:, ci, :],
                                start=(ci == ids[0]), stop=False)
                            if c == 2:
                                oi = oidx[ci]
                                nc.tensor.matmul(
                                    dd[:], vc[:, ci, msl], tda[:, oi, :],
                                    start=(oi == 0), stop=False)
                            nc.tensor.matmul(
                                pp[c][:], vs[:, ci, msl], tpb[:, ci, :],
                                start=False, stop=(ci == ids[-1]))
                            if c == 2:
                                nc.tensor.matmul(
                                    dd[:], vs[:, ci, msl], tdb[:, oi, :],
                                    start=False, stop=(oi == NODD - 1))
                        # A = P0+P2, C = P0-P2; y_j = A+-B / C+-D.  DVE may
                        # read only one PSUM operand, so P0 goes to SBUF
                        # first (on the otherwise-idle scalar engine).
                        p0sb = acp.tile([P, NT], f32, tag="p0", name="p0sb")
                        at = acp.tile([P, NT], f32, tag="A", name="at")
                        ct = acp.tile([P, NT], f32, tag="C", name="ct")
                        nc.scalar.copy(out=p0sb[:], in_=pp[0][:])
                        nc.vector.tensor_tensor(out=at[:], in0=p0sb[:],
                                                in1=pp[1][:], op=add)
                        nc.vector.tensor_tensor(out=ct[:], in0=p0sb[:],
                                                in1=pp[1][:], op=sub)
                        for j, (lhs, ps, op) in enumerate(
                            ((at, pp[2], add), (ct, dd, add),
                             (at, pp[2], sub), (ct, dd, sub))):
                            ot = opool.tile([P, NT], f16, tag=f"y{j}",
                                            name=f"y{j}")
                            nc.vector.tensor_tensor(out=ot[:], in0=lhs[:],
                                                    in1=ps[:], op=op)
                            col = pos * NT + LQ * j
                            eng = nc.scalar if j % 2 == 0 else nc.sync
                            eng.dma_start(outp[:, ms, col : col + NT], ot[:])
    nc.finalize()
    return nc


def _host_prep(x, spectrum, indices):
    x2 = np.asarray(x, dtype=np.float32).reshape(ROWS, IN_F)
    idx = np.asarray(indices, dtype=np.int64)
    s = np.asarray(spectrum, dtype=np.float32)
    a, b = idx[0], idx[1]

    # reference scatter is last-write-wins on duplicate (a,b) pairs
    keys = a * OUT_F + b
    _, first_of_reversed = np.unique(keys[::-1], return_index=True)
    keep = np.zeros(NF, dtype=bool)
    keep[NF - 1 - first_of_reversed] = True
    s_eff = np.where(keep, s, 0.0).astype(np.float32)

    # per-set matmul streams: (n table blocks, xop block offset, 0=cos/1=sin)
    SET_STREAMS = {
        0: ((4, 0, 0), (1, 32, 0), (4, 4, 1)),    # a%4==0: xA0c, xE0, xA0s
        1: ((4, 8, 0), (4, 12, 1)),               # a%4==2: xA2c, xA2s
        2: ((8, 16, 0), (8, 24, 1)),              # a odd:  xmc,  xms
    }
    # chunks: 3x3 (a-fold, b-class) groups, even-a remainders merged per
    # b-class into mixed chunks; even-a chunks first, odd-a last (the xm
    # operands stream later); stage-2 class membership is an explicit list
    afold = [(lambda v: v % 4 == 0, 0, 0), (lambda v: v % 4 == 2, 1, 2),
             (lambda v: v % 2 == 1, 2, 1)]
    bklass = [(lambda v: v % 4 == 0, 0), (lambda v: v % 4 == 2, 2),
              (lambda v: v % 2 == 1, 1)]
    chunk_f, chunk_xsel, desc = [], [], []
    odd_f, odd_xsel, odd_desc = [], [], []

    def mkstreams(xsi):
        st = ()
        for xs in sorted(set(xsi)):
            st = st + SET_STREAMS[xs]
        return st

    for bcl, (bsel, db) in enumerate(bklass):
        pool_f, pool_xs = [], []
        for asel, xsel, da in afold[:2]:
            sel = np.nonzero(asel(a) & bsel(b))[0]
            nfull = len(sel) // P
            for c in range(nfull):
                chunk_f.append(sel[c * P : (c + 1) * P])
                chunk_xsel.append([xsel] * P)
                desc.append((bcl, SET_STREAMS[xsel]))
            pool_f += list(sel[nfull * P :])
            pool_xs += [xsel] * (len(sel) - nfull * P)
        mixed = [(pool_f[c0 : c0 + P], pool_xs[c0 : c0 + P])
                 for c0 in range(0, len(pool_f), P)]
        asel, xsel, da = afold[2]
        sel = np.nonzero(asel(a) & bsel(b))[0]
        nfull = len(sel) // P
        rem = list(sel[nfull * P :])
        # merge the odd-a remainder into a partial even chunk of the SAME
        # b-class (stage-2 only needs b-class purity): saves a whole chunk
        # of stage-2 matmuls at zero stage-1 cost
        if rem and mixed and len(mixed[-1][0]) + len(rem) <= P:
            fi, xsi = mixed.pop()
            fi = fi + rem
            xsi = xsi + [2] * len(rem)
            pad = P - len(fi)
            odd_f.append(np.array(fi + [-1] * pad))
            odd_xsel.append(xsi + [xsi[0]] * pad)
            odd_desc.append((bcl, mkstreams(xsi)))
            rem = []
        for fi, xsi in mixed:
            pad = P - len(fi)
            chunk_f.append(np.array(fi + [-1] * pad))
            chunk_xsel.append(xsi + [xsi[0]] * pad)
            desc.append((bcl, mkstreams(xsi)))
        for c in range(nfull):
            odd_f.append(sel[c * P : (c + 1) * P])
            odd_xsel.append([2] * P)
            odd_desc.append((bcl, SET_STREAMS[2]))
        if rem or (nfull == 0 and not any(d[0] == bcl for d in odd_desc)
                   and not any(d[0] == bcl for d in desc)):
            pad = P - len(rem)
            odd_f.append(np.array(rem + [-1] * pad))
            odd_xsel.append([2] * P)
            odd_desc.append((bcl, SET_STREAMS[2]))
    def ekey(d):
        if d[1] == SET_STREAMS[1]:
            return 0
        if d[1] == SET_STREAMS[0]:
            return 1
        return 2
    ez = sorted(zip(chunk_f, chunk_xsel, desc), key=lambda z: ekey(z[2]))
    chunk_f = [z[0] for z in ez] + odd_f
    chunk_xsel = [z[1] for z in ez] + odd_xsel
    desc = [z[2] for z in ez] + odd_desc
    # padded lane arrays; dummy lanes (f == -1) get parity-consistent a/b
    dummy_b = {0: 0, 1: 2, 2: 1}
    dummy_a = {0: 0, 1: 2, 2: 1}
    NCH = len(desc)
    a2 = np.zeros(NCH * P, np.int64)
    b2 = np.zeros(NCH * P, np.int64)
    s2 = np.zeros(NCH * P, np.float32)
    for i in range(NCH):
        bcl = desc[i][0]
        fi = chunk_f[i]
        for j in range(P):
            if fi[j] >= 0:
                a2[i * P + j] = a[fi[j]]
                b2[i * P + j] = b[fi[j]]
                s2[i * P + j] = s_eff[fi[j]]
            else:
                a2[i * P + j] = dummy_a[chunk_xsel[i][j]]
                b2[i * P + j] = dummy_b[bcl]

    w = 2.0 * np.pi / 4096.0
    # stage-1 tables: per chunk, per stream, cos/sin of the LOCAL operand
    # index; singleton terms live in host-overridden rows (see xop build)
    xgrp = {0: 0, 4: 0, 32: 0, 8: 1, 12: 1, 16: 2, 24: 2}
    t1_parts = []
    for i, (bcl, streams) in enumerate(desc):
        al = a2[i * P : (i + 1) * P]
        mask = None
        xsl = np.array(chunk_xsel[i])
        for nblk, xb0, tgt in streams:
            m = (xsl == xgrp[xb0])[None, :]
            if xb0 == 32:
                blk = np.zeros((P, P), np.float32)
                blk[0] = np.where(m[0], (-1.0) ** ((al // 4) % 2), 0.0)
                t1_parts.append(blk)
                continue
            jj = np.arange(nblk * P)
            ph = (al[None, :] * jj[:, None]) % 4096
            tb = (np.cos(w * ph, dtype=np.float32) if tgt == 0
                  else np.sin(w * ph, dtype=np.float32))
            if tgt == 1 and xb0 == 12:
                tb[0] = (-1.0) ** (((al - 2) // 4) % 2)
            if tgt == 1 and xb0 == 24:
                tb[0] = (-1.0) ** (((al - 1) // 2) % 2)
            t1_parts.append(np.where(m, tb, 0.0).astype(np.float32))
    t1c = np.ascontiguousarray(
        np.concatenate(t1_parts, axis=0).astype(np.float16))

    # stage-2 tables over l' < 1024, s*2^-8 folded in
    ll = np.arange(LQ)
    odd_ids = [i for i, (bcl, _) in enumerate(desc) if bcl == 2]
    NODD = len(odd_ids)
    t2pc = np.zeros((NCH, NPOS, P, NT), np.float32)
    t2ps = np.zeros((NCH, NPOS, P, NT), np.float32)
    t2dc = np.zeros((NODD, NPOS, P, NT), np.float32)
    t2ds = np.zeros((NODD, NPOS, P, NT), np.float32)
    oi = 0
    for i, (bcl, _) in enumerate(desc):
        bl = b2[i * P : (i + 1) * P]
        sc = (s2[i * P : (i + 1) * P] * 2.0 ** -8)[:, None]
        ph = (bl[:, None] * ll[None, :]) % 4096
        cosb = np.cos(w * ph, dtype=np.float32)
        sinb = np.sin(w * ph, dtype=np.float32)
        t2pc[i] = (cosb * sc).reshape(P, NPOS, NT).transpose(1, 0, 2)
        t2ps[i] = (-sinb * sc).reshape(P, NPOS, NT).transpose(1, 0, 2)
        if bcl == 2:
            sg = np.where(bl % 4 == 3, 1.0, -1.0)[:, None]
            t2dc[oi] = (sinb * sc * sg).reshape(P, NPOS, NT).transpose(1, 0, 2)
            t2ds[oi] = (cosb * sc * sg).reshape(P, NPOS, NT).transpose(1, 0, 2)
            oi += 1
    t2pc = np.ascontiguousarray(t2pc.reshape(NCH * NPOS * P, NT).astype(np.float16))
    t2ps = np.ascontiguousarray(t2ps.reshape(NCH * NPOS * P, NT).astype(np.float16))
    t2dc = np.ascontiguousarray(t2dc.reshape(NODD * NPOS * P, NT).astype(np.float16))
    t2ds = np.ascontiguousarray(t2ds.reshape(NODD * NPOS * P, NT).astype(np.float16))

    # folded x operands: parity folds then reflection folds (singletons in
    # slot 0 / the xE0 block, matching the table-row overrides above)
    q0, q1, q2, q3 = (x2[:, i * KQ : (i + 1) * KQ] for i in range(4))
    xA0 = q0 + q1 + q2 + q3
    xA2 = q0 - q1 + q2 - q3
    xm = x2[:, :KH] - x2[:, KH:]
    rA0 = xA0[:, ::-1]
    rA2 = xA2[:, ::-1]
    rm = xm[:, ::-1]
    half = KQ // 2   # 512
    xA0c = np.concatenate([xA0[:, :1], xA0[:, 1:half] + rA0[:, 0:half - 1]], 1)
    xA0s = np.concatenate([np.zeros_like(xA0[:, :1]),
                           xA0[:, 1:half] - rA0[:, 0:half - 1]], 1)
    xA2c = np.concatenate([xA2[:, :1], xA2[:, 1:half] - rA2[:, 0:half - 1]], 1)
    xA2s = np.concatenate([xA2[:, half : half + 1],
                           xA2[:, 1:half] + rA2[:, 0:half - 1]], 1)
    xmc = np.concatenate([xm[:, :1], xm[:, 1:KQ] - rm[:, 0:KQ - 1]], 1)
    xms = np.concatenate([xm[:, KQ : KQ + 1], xm[:, 1:KQ] + rm[:, 0:KQ - 1]], 1)
    xE0 = np.zeros((ROWS, P), np.float32)
    xE0[:, 0] = xA0[:, half]
    xop = np.concatenate(
        [xA0c, xA0s, xA2c, xA2s, xmc, xms, xE0], axis=1).astype(np.float16)
    tabs = {"t1c": t1c, "t2pc": t2pc, "t2ps": t2ps,
            "t2dc": t2dc, "t2ds": t2ds}
    return xop, tabs, tuple(desc)


def kernel(x, spectrum, indices):
    global _NC_CACHE, LAST_RESULTS
    xop, tabs, desc = _host_prep(x, spectrum, indices)

    if _NC_CACHE is None or _NC_CACHE[0] != desc:
        _NC_CACHE = (desc, _build_nc(desc))
    nc = _NC_CACHE[1]

    in_maps = [
        {
            "xop": np.ascontiguousarray(xop[j * M : (j + 1) * M].T),
            **tabs,
        }
        for j in range(N_CORES)
    ]
    res = run_bass_kernel_spmd(nc, in_maps, list(range(N_CORES)))
    LAST_RESULTS = res
    out = np.concatenate(
        [res.results[j]["out"].astype(np.float32) for j in range(N_CORES)], axis=0
    )
    return out.reshape(np.asarray(x).shape[:-1] + (OUT_F,))
